# revision 35
# baseline (speedup 1.0000x reference)
"""Multi-head attention (B=2, S=2048, D=1024, H=16) on 8 TRN2 NeuronCores.

Sharding: 8-way tensor-parallel over heads (2 heads/core), Megatron-style.
Each core:
  - holds X^T (feature-major) for all 4096 flat tokens (bf16),
  - computes Q^T/K^T (feature-major) and V (token-major) for its 2 heads,
  - computes scores S^T = K_h Q_h^T per k-tile with K=64 matmuls row-packed
    into the 128x128 PE array (one per head),
  - softmax: exp on ScalarE (scale=1/8 folded in, no max-subtraction --
    scores are ~N(0, 1/3) so exp is safe), denominator via a ones-column
    appended to V (M=65 matmul), division via fast reciprocal +
    partition_broadcast,
  - AllToAll over all 8 cores redistributes attention outputs from
    head-sharded to token-sharded (each core ends with all 1024 attention
    features for its 512-token chunk),
  - out-projection (full Wo^T) + bias for its token chunk.
Host only pre-transposes/slices inputs and re-assembles output slices.

v2 schedule: the query axis is processed in THREE column phases of widths
[256, 128, 128].  Phase p's AllToAll and out-projection are hidden under
phase p+1's attention compute; only the final 256KB AllToAll plus a
128-column out-projection are exposed at the tail.  The ScalarE exp stream
(~171us total) is the pacing engine, so:
  - the scalar queue carries only the K-weight DMAs (which gate the first
    exp anyway) and then exp instructions exclusively,
  - QKV projection matmuls for batch 1 are emitted interleaved between
    batch-0 attention groups so the PE fills exp-wait slack instead of
    stalling the ACT stream at the batch seam,
  - exp granularity is 1024 free elements per ACTIVATE in every phase
    (phases B/C group 4 k-tiles per exp to amortize the ~480ns
    per-instruction ACT overhead).

PSUM note: a start=True matmul clears has_written bits for its WHOLE 2KB
bank, so the two heads' o-accumulation chains live in separate banks
(h-stride = 512 f32) and the o tile is drained to SBUF immediately after
the accumulation completes (high priority) so a single o buffer suffices.
"""
import numpy as np
import ml_dtypes

import concourse.bass as bass
import concourse.bacc as bacc
import concourse.tile as tile
import concourse.mybir as mybir
from concourse.bass_utils import run_bass_kernel_spmd
from concourse.bass_interp import get_hw_module

NCORES = 8
B, S, D = 2, 2048, 1024
H, HD = 16, 64
T = B * S                 # 4096 flat tokens
HPC = H // NCORES         # 2 heads per core
FPC = HPC * HD            # 128 features per core
TPC = T // NCORES         # 512-token output chunk per core
ND = D // 128             # 8 d-tiles
NTT = T // 128            # 32 token-tiles
NKT = S // 128            # 16 key tiles per batch
NQC = 4                   # 512-token query chunks per batch
SCALE = 1.0 / float(np.sqrt(HD))

PW = [256, 128, 128]      # query-column width per phase
OFF = [0, 256, 384]       # column offset per phase
KG = [2, 4, 4]            # k-tiles grouped per exp (free = 2*KG*PW = 1024)

BF16 = mybir.dt.bfloat16
F32 = mybir.dt.float32


def build():
    nc = bacc.Bacc("TRN2", target_bir_lowering=False, debug=False,
                   num_devices=NCORES)
    xt = nc.dram_tensor("xt", [D, T], BF16, kind="ExternalInput").ap()
    wqt = nc.dram_tensor("wqt", [D, FPC], BF16, kind="ExternalInput").ap()
    wkt = nc.dram_tensor("wkt", [D, FPC], BF16, kind="ExternalInput").ap()
    wvt = nc.dram_tensor("wvt", [D, FPC], BF16, kind="ExternalInput").ap()
    wot = nc.dram_tensor("wot", [D, D], BF16, kind="ExternalInput").ap()
    bq = nc.dram_tensor("bq", [FPC, 1], F32, kind="ExternalInput").ap()
    bk = nc.dram_tensor("bk", [FPC, 1], F32, kind="ExternalInput").ap()
    bv_row = nc.dram_tensor("bv_row", [1, FPC], BF16, kind="ExternalInput").ap()
    bo = nc.dram_tensor("bo", [D, 1], F32, kind="ExternalInput").ap()
    out_t = nc.dram_tensor("out_t", [D, TPC], F32, kind="ExternalOutput").ap()

    with tile.TileContext(nc) as tc:
        with (
            tc.tile_pool(name="wts", bufs=1) as wts,
            tc.tile_pool(name="acts", bufs=1) as acts,
            tc.tile_pool(name="p_pool", bufs=6) as p_pool,
            tc.tile_pool(name="div_pool", bufs=10) as div_pool,
            tc.tile_pool(name="fin_pool", bufs=2) as fin_pool,
            tc.tile_pool(name="s_ps_pool", bufs=2, space="PSUM") as s_ps_pool,
            tc.tile_pool(name="o_ps_pool", bufs=1, space="PSUM") as o_ps_pool,
            tc.tile_pool(name="mm_ps", bufs=2, space="PSUM") as mm_ps,
            tc.tile_pool(name="dram", bufs=1, space="DRAM") as dram,
        ):
            # ---- ACT warm-up: table load + one tiny exp before anything else
            # ---- on the scalar queue.
            warm_sb = wts.tile([1, 8], F32, name="warm_sb")
            nc.vector.memset(warm_sb[:], 0.0)
            nc.scalar.activation(warm_sb[:], warm_sb[:],
                                 mybir.ActivationFunctionType.Exp, scale=1.0)

            # ---- DMAs, priority order: K-path data first (wk, bk, xt cols
            # ---- 0:1024), then Q weights, V weights, rest of batch 0,
            # ---- batch 1, Wo last.
            xt_sb = acts.tile([128, ND, T], BF16, name="xt_sb")
            wq_sb = wts.tile([128, ND, FPC], BF16, name="wq_sb")
            wk_sb = wts.tile([128, ND, FPC], BF16, name="wk_sb")
            wv_sb = wts.tile([128, ND, FPC], BF16, name="wv_sb")
            bq_sb = wts.tile([FPC, 1], F32, name="bq_sb")
            bk_sb = wts.tile([FPC, 1], F32, name="bk_sb")
            bv_row_sb = wts.tile([1, FPC], BF16, name="bv_row_sb")

            # first xt wave (tokens 0:256 gate the first K/Q projection)
            # split over sync + gpsimd so both DMA paths pull in parallel;
            # K/Q weights on the scalar queue (they gate the first exp anyway)
            for d in range(ND):
                q = nc.sync if d < 4 else nc.gpsimd
                q.dma_start(out=xt_sb[:, d, 0:256], in_=xt[128 * d:128 * (d + 1), 0:256])
            for d in range(ND):
                q = nc.sync if d < 4 else nc.gpsimd
                q.dma_start(out=xt_sb[:, d, 256:512], in_=xt[128 * d:128 * (d + 1), 256:512])
            for d in range(ND):
                nc.scalar.dma_start(out=wk_sb[:, d, :], in_=wkt[128 * d:128 * (d + 1), :])
            nc.scalar.dma_start(out=bk_sb[:], in_=bk[:])
            # pre-warm the collective path: a tiny dummy AllToAll absorbs the
            # one-time ~50us first-collective setup cost
            cc_wu_in = dram.tile([NCORES, 64], BF16, name="cc_wu_in")
            cc_wu_out = dram.tile([NCORES, 64], BF16, name="cc_wu_out")
            nc.gpsimd.collective_compute(
                "AllToAll", mybir.AluOpType.bypass,
                replica_groups=[list(range(NCORES))],
                ins=[cc_wu_in.opt()], outs=[cc_wu_out.opt()])
            for d in range(ND):
                nc.gpsimd.dma_start(out=wq_sb[:, d, :], in_=wqt[128 * d:128 * (d + 1), :])
            nc.gpsimd.dma_start(out=bq_sb[:], in_=bq[:])
            for d in range(ND):
                nc.sync.dma_start(out=wv_sb[:, d, :], in_=wvt[128 * d:128 * (d + 1), :])
            nc.sync.dma_start(out=bv_row_sb[:], in_=bv_row[:])
            for d in range(ND):
                q = nc.sync if d < 4 else nc.gpsimd
                q.dma_start(out=xt_sb[:, d, 512:1024], in_=xt[128 * d:128 * (d + 1), 512:1024])
            bv_bc = wts.tile([128, FPC], BF16, name="bv_bc")
            nc.gpsimd.partition_broadcast(bv_bc[:], bv_row_sb[:])
            for d in range(ND):
                q = nc.sync if d < 4 else nc.gpsimd
                q.dma_start(out=xt_sb[:, d, 1024:2048], in_=xt[128 * d:128 * (d + 1), 1024:2048])
            # batch 1
            for d in range(ND):
                q = nc.sync if d < 4 else nc.gpsimd
                q.dma_start(out=xt_sb[:, d, S:T], in_=xt[128 * d:128 * (d + 1), S:T])
            # out-projection weights (needed from ~mid-run)
            wot_sb = wts.tile([128, ND, D], BF16, name="wot_sb")
            bo_sb = wts.tile([128, ND], F32, name="bo_sb")
            for d in range(ND):
                nc.gpsimd.dma_start(out=wot_sb[:, d, :], in_=wot[128 * d:128 * (d + 1), :])
            for m in range(ND):
                nc.gpsimd.dma_start(out=bo_sb[:, m:m + 1], in_=bo[128 * m:128 * (m + 1), :])

            # ---- warm up the PE HAM clock gate (needs ~3.4us of sustained
            # ---- matmul activity to unthrottle 1.2 -> 2.4 GHz)
            wmm_sb = wts.tile([128, 512], BF16, name="wmm_sb")
            nc.vector.memset(wmm_sb[:], 0.0)
            warm_ps = mm_ps.tile([128, 512], F32, tag="mm_ps", name="warm_ps")
            for i in range(24):
                nc.tensor.matmul(warm_ps[:], wmm_sb[:, 0:128], wmm_sb[:, 0:512],
                                 start=(i == 0), stop=(i == 23))
            nc.vector.tensor_copy(wmm_sb[0:1, 0:1], warm_ps[0:1, 0:1])

            qt_sb = acts.tile([FPC, T], BF16, name="qt_sb")
            kt_sb = acts.tile([FPC, T], BF16, name="kt_sb")
            v_sb = acts.tile([128, NTT, HPC, HD + 1], BF16, name="v_sb")
            nc.vector.memset(v_sb[:, :, :, HD:HD + 1], 1.0)

            def proj_qk(w_sb, b_sb, dst, tch):
                ps = mm_ps.tile([128, 512], F32, tag="mm_ps", name="proj_ps")
                for d in range(ND):
                    nc.tensor.matmul(
                        ps[:], w_sb[:, d, :],
                        xt_sb[:, d, 512 * tch:512 * (tch + 1)],
                        start=(d == 0), stop=(d == ND - 1))
                nc.vector.tensor_scalar_add(
                    dst[:, 512 * tch:512 * (tch + 1)], ps[:], b_sb[:])

            def proj_qk_w(w_sb, b_sb, dst, t0, tw):
                # narrow-width projection for startup ramp (tokens t0:t0+tw)
                ps = mm_ps.tile([128, 512], F32, tag="mm_ps", name="proj_ps")
                for d in range(ND):
                    nc.tensor.matmul(
                        ps[:, 0:tw], w_sb[:, d, :],
                        xt_sb[:, d, t0:t0 + tw],
                        start=(d == 0), stop=(d == ND - 1))
                nc.vector.tensor_scalar_add(
                    dst[:, t0:t0 + tw], ps[:, 0:tw], b_sb[:])

            def proj_qk_split(w_sb, b_sb, dst, tch):
                # returns two half-chain emitters so a projection can spread
                # over two attention groups' exp-wait slack
                ps = mm_ps.tile([128, 512], F32, tag="mm_ps", name="proj_ps")

                def part1():
                    for d in range(4):
                        nc.tensor.matmul(
                            ps[:], w_sb[:, d, :],
                            xt_sb[:, d, 512 * tch:512 * (tch + 1)],
                            start=(d == 0), stop=False)

                def part2():
                    for d in range(4, ND):
                        nc.tensor.matmul(
                            ps[:], w_sb[:, d, :],
                            xt_sb[:, d, 512 * tch:512 * (tch + 1)],
                            start=False, stop=(d == ND - 1))
                    nc.vector.tensor_scalar_add(
                        dst[:, 512 * tch:512 * (tch + 1)], ps[:], b_sb[:])
                return part1, part2

            def proj_v(tt):
                ps = mm_ps.tile([128, FPC], F32, tag="mm_ps", name="v_ps")
                for d in range(ND):
                    nc.tensor.matmul(
                        ps[:], xt_sb[:, d, 128 * tt:128 * (tt + 1)],
                        wv_sb[:, d, :],
                        start=(d == 0), stop=(d == ND - 1))
                nc.vector.tensor_tensor(
                    v_sb[:, tt, :, 0:HD],
                    ps.rearrange("p (h f) -> p h f", h=HPC),
                    bv_bc.rearrange("p (h f) -> p h f", h=HPC),
                    mybir.AluOpType.add)

            # per-phase A2A bounce buffers
            cc_in = [dram.tile([NCORES * FPC, PW[p]], BF16, name=f"cc_in{p}")
                     for p in range(len(PW))]
            cc_out = [dram.tile([NCORES * FPC, PW[p]], BF16, name=f"cc_out{p}")
                      for p in range(len(PW))]
            at_full = [acts.tile([128, NCORES, PW[p]], BF16, name=f"at_full{p}")
                       for p in range(len(PW))]

            def attention(p, b, qc, fillers=(), hp_div=False, o_delay=0):
                """One (batch, query-chunk) call for column phase p.

                fillers: list of (group_idx, emit_fn) -- extra PE work emitted
                right after that group's score matmuls so it executes in the
                exp-wait slack without delaying the ACT stream.
                """
                w = PW[p]
                kg = KG[p]
                ngrp = NKT // kg
                q0 = 2048 * b + 512 * qc + OFF[p]
                # h-stride = 512 f32 = one full PSUM bank per head: a
                # start=True matmul clears has_written for its whole bank.
                o_ps = o_ps_pool.tile([128, HPC, 512], F32, name="o_ps")
                # o_delay: hold back the first groups' o-chains (keeps early
                # scores from sitting behind V-DMA-gated o-matmuls in the PE
                # queue during the startup ramp); accumulation order preserved.
                held = []
                for g in range(ngrp):
                    s_ps = s_ps_pool.tile([128, HPC, kg, w], F32, name="s_ps")
                    for ki in range(kg):
                        k = g * kg + ki
                        k0 = 2048 * b + 128 * k
                        for h in range(HPC):
                            nc.tensor.matmul(
                                s_ps[:, h, ki, :],
                                kt_sb[64 * h:64 * (h + 1), k0:k0 + 128],
                                qt_sb[64 * h:64 * (h + 1), q0:q0 + w],
                                start=True, stop=True)
                    for (fg, fn) in fillers:
                        if fg == g:
                            fn()
                    p_t = p_pool.tile([128, HPC, kg, w], BF16, name="p_t")
                    nc.scalar.activation(
                        p_t[:], s_ps[:],
                        mybir.ActivationFunctionType.Exp, scale=SCALE)

                    def emit_o(g, p_t):
                        for ki in range(kg):
                            k = g * kg + ki
                            for h in range(HPC):
                                nc.tensor.matmul(
                                    o_ps[0:HD + 1, h, 0:w],
                                    v_sb[:, NKT * b + k, h, :],
                                    p_t[:, h, ki, :],
                                    start=(k == 0), stop=(k == NKT - 1))
                    if g < o_delay:
                        held.append((g, p_t))
                    else:
                        for (hg, hp) in held:
                            emit_o(hg, hp)
                        held = []
                        emit_o(g, p_t)
                j = NQC * b + qc
                from contextlib import nullcontext
                for h in range(HPC):
                    prio = tc.high_priority() if hp_div else nullcontext()
                    with prio:
                        # drain o out of PSUM first so the single o buffer
                        # frees before the next call's accumulation
                        ovs = div_pool.tile([HD + 1, w], F32, name="ovs")
                        nc.vector.tensor_copy(ovs[:], o_ps[0:HD + 1, h, 0:w])
                        den_sb = div_pool.tile([1, w], F32, name="den_sb")
                        nc.vector.tensor_copy(den_sb[:], ovs[HD:HD + 1, :])
                        recip = div_pool.tile([1, w], F32, name="recip")
                        nc.vector.reciprocal_approx_fast(recip[:], den_sb[:])
                        rb = div_pool.tile([HD, w], F32, name="rb")
                        nc.gpsimd.partition_broadcast(rb[:], recip[:])
                        avs = div_pool.tile([HD, w], BF16, name="avs")
                        nc.vector.tensor_tensor(
                            avs[:], ovs[0:HD, :], rb[:],
                            mybir.AluOpType.mult)
                        # gpsimd queue: the sync queue can head-of-line block
                        # for the full collective duration behind hoisted
                        # at_full loads -- the division path must not sit
                        # behind that, whatever the collective's speed
                        nc.gpsimd.dma_start(
                            out=cc_in[p][FPC * j + HD * h: FPC * j + HD * (h + 1), :],
                            in_=avs[:])

            def do_a2a(p):
                nc.gpsimd.collective_compute(
                    "AllToAll", mybir.AluOpType.bypass,
                    replica_groups=[list(range(NCORES))],
                    ins=[cc_in[p].opt()], outs=[cc_out[p].opt()])

            def load_at_full(p):
                for jj in range(NCORES):
                    nc.sync.dma_start(
                        out=at_full[p][:, jj, :],
                        in_=cc_out[p][FPC * jj:FPC * (jj + 1), :])

            def out_proj_m(p, m):
                w = PW[p]
                ps = mm_ps.tile([128, 512], F32, tag="mm_ps", name="f_ps")
                for d in range(ND):
                    nc.tensor.matmul(
                        ps[:, 0:w], wot_sb[:, d, 128 * m:128 * (m + 1)],
                        at_full[p][:, d, :],
                        start=(d == 0), stop=(d == ND - 1))
                fin = fin_pool.tile([128, w], F32, name="fin")
                nc.vector.tensor_scalar_add(fin[:], ps[:, 0:w], bo_sb[:, m:m + 1])
                q = nc.sync if m % 2 == 0 else nc.gpsimd
                q.dma_start(
                    out=out_t[128 * m:128 * (m + 1), OFF[p]:OFF[p] + w],
                    in_=fin[:])

            # ================= phase A (cols 0:256) =================
            # upfront: only tokens 0:256 of K and Q -- the minimum for the
            # first score group; everything else arrives just-in-time
            proj_qk_w(wk_sb, bk_sb, kt_sb, 0, 256)
            proj_qk_w(wq_sb, bq_sb, qt_sb, 0, 256)

            # call (0,0): K chunks / Q remainder / V tiles just-in-time
            k1a, k1b = proj_qk_split(wk_sb, bk_sb, kt_sb, 1)
            k2a, k2b = proj_qk_split(wk_sb, bk_sb, kt_sb, 2)
            k3a, k3b = proj_qk_split(wk_sb, bk_sb, kt_sb, 3)
            attention(0, 0, 0, o_delay=4, fillers=[
                # with o_delay=4 the V tiles are not consumed until group 4's
                # flush, so the exp stream only gates on the K/Q chain here
                (0, lambda: (proj_qk_w(wk_sb, bk_sb, kt_sb, 256, 256),
                             proj_qk_w(wq_sb, bq_sb, qt_sb, 256, 256))),
                (1, lambda: (k1a(), k1b())),
                (2, lambda: (k2a(), proj_v(0), proj_v(1), proj_v(2))),
                (3, lambda: (k2b(), proj_v(3), proj_v(4), proj_v(5))),
                (4, lambda: (k3a(), proj_v(6), proj_v(7), proj_v(8))),
                (5, lambda: (k3b(), proj_v(9), proj_v(10), proj_v(11))),
                (6, lambda: (proj_v(12), proj_v(13), proj_v(14))),
                (7, lambda: (proj_v(15), proj_qk(wq_sb, bq_sb, qt_sb, 1))),
            ])
            # calls (0,1)-(0,3): spread batch-1 K/Q/V projections, with
            # Q/K chains split in half so no group slot exceeds its slack
            q2a, q2b = proj_qk_split(wq_sb, bq_sb, qt_sb, 2)
            k4a, k4b = proj_qk_split(wk_sb, bk_sb, kt_sb, 4)
            attention(0, 0, 1, fillers=[
                (0, q2a), (1, q2b),
                (2, lambda: (k4a(), proj_v(16))), (3, lambda: (k4b(), proj_v(17))),
                (4, lambda: proj_v(18)), (5, lambda: proj_v(19)),
                (6, lambda: proj_v(20)), (7, lambda: proj_v(21)),
            ])
            q3a, q3b = proj_qk_split(wq_sb, bq_sb, qt_sb, 3)
            k5a, k5b = proj_qk_split(wk_sb, bk_sb, kt_sb, 5)
            attention(0, 0, 2, fillers=[
                (0, q3a), (1, q3b),
                (2, lambda: (k5a(), proj_v(22))), (3, lambda: (k5b(), proj_v(23))),
                (4, lambda: proj_v(24)), (5, lambda: proj_v(25)),
                (6, lambda: proj_v(26)), (7, lambda: proj_v(27)),
            ])
            k6a, k6b = proj_qk_split(wk_sb, bk_sb, kt_sb, 6)
            k7a, k7b = proj_qk_split(wk_sb, bk_sb, kt_sb, 7)
            q4a, q4b = proj_qk_split(wq_sb, bq_sb, qt_sb, 4)
            attention(0, 0, 3, fillers=[
                (0, k6a), (1, k6b),
                (2, lambda: (k7a(), proj_v(28))), (3, lambda: (k7b(), proj_v(29))),
                (4, lambda: (q4a(), proj_v(30))), (5, lambda: (q4b(), proj_v(31))),
            ])
            q5a, q5b = proj_qk_split(wq_sb, bq_sb, qt_sb, 5)
            attention(0, 1, 0, fillers=[(0, q5a), (1, q5b)])
            q6a, q6b = proj_qk_split(wq_sb, bq_sb, qt_sb, 6)
            attention(0, 1, 1, fillers=[(0, q6a), (1, q6b)])
            q7a, q7b = proj_qk_split(wq_sb, bq_sb, qt_sb, 7)
            attention(0, 1, 2, fillers=[(0, q7a), (1, q7b)])
            attention(0, 1, 3)
            do_a2a(0)

            # ================= phase B (cols 256:384) =================
            # a2a(0) runs under the first ~half of this phase; out-projection
            # of phase A is emitted in the second half (after it is surely
            # complete) so the PE never head-of-line blocks on the collective.
            attention(1, 0, 0)
            attention(1, 0, 1)
            attention(1, 0, 2)
            attention(1, 0, 3)
            attention(1, 1, 0)
            attention(1, 1, 1)
            attention(1, 1, 2)
            attention(1, 1, 3)
            do_a2a(1)

            # ================= phase C (cols 384:512) =================
            load_at_full(0)
            attention(2, 0, 0, fillers=[
                (0, lambda: out_proj_m(0, 0)), (2, lambda: out_proj_m(0, 1)),
            ])
            attention(2, 0, 1, fillers=[
                (0, lambda: out_proj_m(0, 2)), (2, lambda: out_proj_m(0, 3)),
            ])
            attention(2, 0, 2, fillers=[
                (0, lambda: out_proj_m(0, 4)), (2, lambda: out_proj_m(0, 5)),
            ])
            attention(2, 0, 3, fillers=[
                (0, lambda: out_proj_m(0, 6)), (2, lambda: out_proj_m(0, 7)),
            ])
            load_at_full(1)
            attention(2, 1, 0, fillers=[
                (0, lambda: out_proj_m(1, 0)), (2, lambda: out_proj_m(1, 1)),
            ])
            attention(2, 1, 1, fillers=[
                (0, lambda: out_proj_m(1, 2)), (2, lambda: out_proj_m(1, 3)),
            ])
            attention(2, 1, 2, fillers=[
                (0, lambda: out_proj_m(1, 4)), (2, lambda: out_proj_m(1, 5)),
            ])
            attention(2, 1, 3, fillers=[
                (0, lambda: out_proj_m(1, 6)), (2, lambda: out_proj_m(1, 7)),
            ])
            do_a2a(2)

            # ================= exposed tail =================
            # keep the PE busy (HAM warm) across the final collective window
            tail_ps = mm_ps.tile([128, 512], F32, tag="mm_ps", name="tail_ps")
            for i in range(60):
                nc.tensor.matmul(tail_ps[:, 0:256], wmm_sb[:, 0:128],
                                 wmm_sb[:, 0:256],
                                 start=(i == 0), stop=(i == 59))
            for jj in range(NCORES):
                q = nc.sync if jj < 4 else nc.gpsimd
                q.dma_start(
                    out=at_full[2][:, jj, :],
                    in_=cc_out[2][FPC * jj:FPC * (jj + 1), :])
            for m in range(ND):
                out_proj_m(2, m)

    nc.compile()
    nc.m = get_hw_module(nc.m)
    return nc


_NC_CACHE = None


def _get_nc():
    global _NC_CACHE
    if _NC_CACHE is None:
        _NC_CACHE = build()
    return _NC_CACHE


def _make_in_maps(x, Wq, bq, Wk, bk, Wv, bv, Wo, bo):
    bf16 = ml_dtypes.bfloat16
    x = np.asarray(x, np.float32)
    xt = np.ascontiguousarray(x.reshape(T, D).T).astype(bf16)
    wot = np.ascontiguousarray(np.asarray(Wo, np.float32).T).astype(bf16)
    bo_col = np.asarray(bo, np.float32).reshape(D, 1)
    in_maps = []
    for c in range(NCORES):
        hs = slice(FPC * c, FPC * (c + 1))
        in_maps.append({
            "xt": xt,
            "wqt": np.ascontiguousarray(np.asarray(Wq, np.float32)[hs, :].T).astype(bf16),
            "wkt": np.ascontiguousarray(np.asarray(Wk, np.float32)[hs, :].T).astype(bf16),
            "wvt": np.ascontiguousarray(np.asarray(Wv, np.float32)[hs, :].T).astype(bf16),
            "wot": wot,
            "bq": np.asarray(bq, np.float32)[hs].reshape(FPC, 1),
            "bk": np.asarray(bk, np.float32)[hs].reshape(FPC, 1),
            "bv_row": np.asarray(bv, np.float32)[hs].reshape(1, FPC).astype(bf16),
            "bo": bo_col,
        })
    return in_maps


def run_on_hw(in_maps, trace=False):
    nc = _get_nc()
    return run_bass_kernel_spmd(nc, in_maps, list(range(NCORES)), trace=trace)


def _assemble(results):
    out = np.empty((T, D), np.float32)
    for c in range(NCORES):
        out[TPC * c:TPC * (c + 1), :] = results[c]["out_t"].T
    return out.reshape(B, S, D)


def kernel(x, Wq, bq, Wk, bk, Wv, bv, Wo, bo):
    in_maps = _make_in_maps(x, Wq, bq, Wk, bk, Wv, bv, Wo, bo)
    res = run_on_hw(in_maps, trace=False)
    return _assemble(res.results)


# revision 36
# speedup vs baseline: 1.0523x; 1.0523x over previous
"""Multi-head attention (B=2, S=2048, D=1024, H=16) on 8 TRN2 NeuronCores.

Sharding: 8-way tensor-parallel over heads (2 heads/core), Megatron-style.
Each core:
  - holds X^T (feature-major) for all 4096 flat tokens (bf16),
  - computes Q^T/K^T (feature-major) and V (token-major) for its 2 heads,
  - computes scores S^T = K_h Q_h^T per k-tile with K=64 matmuls row-packed
    into the 128x128 PE array (one per head),
  - softmax: exp on ScalarE (scale=1/8 folded in, no max-subtraction --
    scores are ~N(0, 1/3) so exp is safe), denominator via a ones-column
    appended to V (M=65 matmul), division via fast reciprocal +
    partition_broadcast,
  - AllToAll over all 8 cores redistributes attention outputs from
    head-sharded to token-sharded (each core ends with all 1024 attention
    features for its 512-token chunk),
  - out-projection (full Wo^T) + bias for its token chunk.
Host only pre-transposes/slices inputs and re-assembles output slices.

v2 schedule: the query axis is processed in THREE column phases of widths
[256, 128, 128].  Phase p's AllToAll and out-projection are hidden under
phase p+1's attention compute; only the final 256KB AllToAll plus a
128-column out-projection are exposed at the tail.  The ScalarE exp stream
(~171us total) is the pacing engine, so:
  - the scalar queue carries only the K-weight DMAs (which gate the first
    exp anyway) and then exp instructions exclusively,
  - QKV projection matmuls for batch 1 are emitted interleaved between
    batch-0 attention groups so the PE fills exp-wait slack instead of
    stalling the ACT stream at the batch seam,
  - exp granularity is 1024 free elements per ACTIVATE in every phase
    (phases B/C group 4 k-tiles per exp to amortize the ~480ns
    per-instruction ACT overhead).

PSUM note: a start=True matmul clears has_written bits for its WHOLE 2KB
bank, so the two heads' o-accumulation chains live in separate banks
(h-stride = 512 f32) and the o tile is drained to SBUF immediately after
the accumulation completes (high priority) so a single o buffer suffices.
"""
import numpy as np
import ml_dtypes

import concourse.bass as bass
import concourse.bacc as bacc
import concourse.tile as tile
import concourse.mybir as mybir
from concourse.bass_utils import run_bass_kernel_spmd
from concourse.bass_interp import get_hw_module

NCORES = 8
B, S, D = 2, 2048, 1024
H, HD = 16, 64
T = B * S                 # 4096 flat tokens
HPC = H // NCORES         # 2 heads per core
FPC = HPC * HD            # 128 features per core
TPC = T // NCORES         # 512-token output chunk per core
ND = D // 128             # 8 d-tiles
NTT = T // 128            # 32 token-tiles
NKT = S // 128            # 16 key tiles per batch
NQC = 4                   # 512-token query chunks per batch
SCALE = 1.0 / float(np.sqrt(HD))

PW = [256, 128, 128]      # query-column width per phase
OFF = [0, 256, 384]       # column offset per phase
KG = [2, 4, 4]            # k-tiles grouped per exp (free = 2*KG*PW = 1024)

BF16 = mybir.dt.bfloat16
F32 = mybir.dt.float32


def build():
    nc = bacc.Bacc("TRN2", target_bir_lowering=False, debug=False,
                   num_devices=NCORES)
    xt = nc.dram_tensor("xt", [D, T], BF16, kind="ExternalInput").ap()
    wqt = nc.dram_tensor("wqt", [D, FPC], BF16, kind="ExternalInput").ap()
    wkt = nc.dram_tensor("wkt", [D, FPC], BF16, kind="ExternalInput").ap()
    wvt = nc.dram_tensor("wvt", [D, FPC], BF16, kind="ExternalInput").ap()
    wot = nc.dram_tensor("wot", [D, D], BF16, kind="ExternalInput").ap()
    bq = nc.dram_tensor("bq", [FPC, 1], F32, kind="ExternalInput").ap()
    bk = nc.dram_tensor("bk", [FPC, 1], F32, kind="ExternalInput").ap()
    bv_row = nc.dram_tensor("bv_row", [1, FPC], BF16, kind="ExternalInput").ap()
    bo = nc.dram_tensor("bo", [D, 1], F32, kind="ExternalInput").ap()
    out_t = nc.dram_tensor("out_t", [D, TPC], F32, kind="ExternalOutput").ap()

    with tile.TileContext(nc) as tc:
        with (
            tc.tile_pool(name="wts", bufs=1) as wts,
            tc.tile_pool(name="acts", bufs=1) as acts,
            tc.tile_pool(name="p_pool", bufs=6) as p_pool,
            tc.tile_pool(name="div_pool", bufs=10) as div_pool,
            tc.tile_pool(name="fin_pool", bufs=2) as fin_pool,
            tc.tile_pool(name="s_ps_pool", bufs=2, space="PSUM") as s_ps_pool,
            tc.tile_pool(name="o_ps_pool", bufs=1, space="PSUM") as o_ps_pool,
            tc.tile_pool(name="mm_ps", bufs=2, space="PSUM") as mm_ps,
            tc.tile_pool(name="dram", bufs=1, space="DRAM") as dram,
        ):
            # ---- ACT warm-up: table load + one tiny exp before anything else
            # ---- on the scalar queue.
            warm_sb = wts.tile([1, 8], F32, name="warm_sb")
            nc.vector.memset(warm_sb[:], 0.0)
            nc.scalar.activation(warm_sb[:], warm_sb[:],
                                 mybir.ActivationFunctionType.Exp, scale=1.0)

            # ---- DMAs, priority order: K-path data first (wk, bk, xt cols
            # ---- 0:1024), then Q weights, V weights, rest of batch 0,
            # ---- batch 1, Wo last.
            xt_sb = acts.tile([128, ND, T], BF16, name="xt_sb")
            wq_sb = wts.tile([128, ND, FPC], BF16, name="wq_sb")
            wk_sb = wts.tile([128, ND, FPC], BF16, name="wk_sb")
            wv_sb = wts.tile([128, ND, FPC], BF16, name="wv_sb")
            bq_sb = wts.tile([FPC, 1], F32, name="bq_sb")
            bk_sb = wts.tile([FPC, 1], F32, name="bk_sb")
            bv_row_sb = wts.tile([1, FPC], BF16, name="bv_row_sb")

            # first xt wave (tokens 0:256 gate the first K/Q projection)
            # split over sync + gpsimd so both DMA paths pull in parallel;
            # K/Q weights on the scalar queue (they gate the first exp anyway)
            for d in range(ND):
                q = nc.sync if d < 4 else nc.gpsimd
                q.dma_start(out=xt_sb[:, d, 0:256], in_=xt[128 * d:128 * (d + 1), 0:256])
            for d in range(ND):
                q = nc.sync if d < 4 else nc.gpsimd
                q.dma_start(out=xt_sb[:, d, 256:512], in_=xt[128 * d:128 * (d + 1), 256:512])
            for d in range(ND):
                nc.scalar.dma_start(out=wk_sb[:, d, :], in_=wkt[128 * d:128 * (d + 1), :])
            nc.scalar.dma_start(out=bk_sb[:], in_=bk[:])
            # pre-warm the collective path: a tiny dummy AllToAll absorbs the
            # one-time ~50us first-collective setup cost
            cc_wu_in = dram.tile([NCORES, 64], BF16, name="cc_wu_in")
            cc_wu_out = dram.tile([NCORES, 64], BF16, name="cc_wu_out")
            nc.gpsimd.collective_compute(
                "AllToAll", mybir.AluOpType.bypass,
                replica_groups=[list(range(NCORES))],
                ins=[cc_wu_in.opt()], outs=[cc_wu_out.opt()])
            for d in range(ND):
                nc.gpsimd.dma_start(out=wq_sb[:, d, :], in_=wqt[128 * d:128 * (d + 1), :])
            nc.gpsimd.dma_start(out=bq_sb[:], in_=bq[:])
            for d in range(ND):
                nc.sync.dma_start(out=wv_sb[:, d, :], in_=wvt[128 * d:128 * (d + 1), :])
            nc.sync.dma_start(out=bv_row_sb[:], in_=bv_row[:])
            for d in range(ND):
                q = nc.sync if d < 4 else nc.gpsimd
                q.dma_start(out=xt_sb[:, d, 512:1024], in_=xt[128 * d:128 * (d + 1), 512:1024])
            bv_bc = wts.tile([128, FPC], BF16, name="bv_bc")
            nc.gpsimd.partition_broadcast(bv_bc[:], bv_row_sb[:])
            for d in range(ND):
                q = nc.sync if d < 4 else nc.gpsimd
                q.dma_start(out=xt_sb[:, d, 1024:2048], in_=xt[128 * d:128 * (d + 1), 1024:2048])
            # batch 1
            for d in range(ND):
                q = nc.sync if d < 4 else nc.gpsimd
                q.dma_start(out=xt_sb[:, d, S:T], in_=xt[128 * d:128 * (d + 1), S:T])
            # out-projection weights (needed from ~mid-run)
            wot_sb = wts.tile([128, ND, D], BF16, name="wot_sb")
            bo_sb = wts.tile([128, ND], F32, name="bo_sb")
            for d in range(ND):
                nc.gpsimd.dma_start(out=wot_sb[:, d, :], in_=wot[128 * d:128 * (d + 1), :])
            for m in range(ND):
                nc.gpsimd.dma_start(out=bo_sb[:, m:m + 1], in_=bo[128 * m:128 * (m + 1), :])

            # ---- warm up the PE HAM clock gate (needs ~3.4us of sustained
            # ---- matmul activity to unthrottle 1.2 -> 2.4 GHz)
            wmm_sb = wts.tile([128, 512], BF16, name="wmm_sb")
            nc.vector.memset(wmm_sb[:], 0.0)
            warm_ps = mm_ps.tile([128, 512], F32, tag="mm_ps", name="warm_ps")
            for i in range(24):
                nc.tensor.matmul(warm_ps[:], wmm_sb[:, 0:128], wmm_sb[:, 0:512],
                                 start=(i == 0), stop=(i == 23))
            nc.vector.tensor_copy(wmm_sb[0:1, 0:1], warm_ps[0:1, 0:1])

            qt_sb = acts.tile([FPC, T], BF16, name="qt_sb")
            kt_sb = acts.tile([FPC, T], BF16, name="kt_sb")
            v_sb = acts.tile([128, NTT, HPC, HD + 1], BF16, name="v_sb")
            nc.vector.memset(v_sb[:, :, :, HD:HD + 1], 1.0)

            def proj_qk(w_sb, b_sb, dst, tch):
                ps = mm_ps.tile([128, 512], F32, tag="mm_ps", name="proj_ps")
                for d in range(ND):
                    nc.tensor.matmul(
                        ps[:], w_sb[:, d, :],
                        xt_sb[:, d, 512 * tch:512 * (tch + 1)],
                        start=(d == 0), stop=(d == ND - 1))
                nc.vector.tensor_scalar_add(
                    dst[:, 512 * tch:512 * (tch + 1)], ps[:], b_sb[:])

            def proj_qk_w(w_sb, b_sb, dst, t0, tw):
                # narrow-width projection for startup ramp (tokens t0:t0+tw)
                ps = mm_ps.tile([128, 512], F32, tag="mm_ps", name="proj_ps")
                for d in range(ND):
                    nc.tensor.matmul(
                        ps[:, 0:tw], w_sb[:, d, :],
                        xt_sb[:, d, t0:t0 + tw],
                        start=(d == 0), stop=(d == ND - 1))
                nc.vector.tensor_scalar_add(
                    dst[:, t0:t0 + tw], ps[:, 0:tw], b_sb[:])

            def proj_qk_split(w_sb, b_sb, dst, tch):
                # returns two half-chain emitters so a projection can spread
                # over two attention groups' exp-wait slack
                ps = mm_ps.tile([128, 512], F32, tag="mm_ps", name="proj_ps")

                def part1():
                    for d in range(4):
                        nc.tensor.matmul(
                            ps[:], w_sb[:, d, :],
                            xt_sb[:, d, 512 * tch:512 * (tch + 1)],
                            start=(d == 0), stop=False)

                def part2():
                    for d in range(4, ND):
                        nc.tensor.matmul(
                            ps[:], w_sb[:, d, :],
                            xt_sb[:, d, 512 * tch:512 * (tch + 1)],
                            start=False, stop=(d == ND - 1))
                    nc.vector.tensor_scalar_add(
                        dst[:, 512 * tch:512 * (tch + 1)], ps[:], b_sb[:])
                return part1, part2

            def proj_v(tt):
                ps = mm_ps.tile([128, FPC], F32, tag="mm_ps", name="v_ps")
                for d in range(ND):
                    nc.tensor.matmul(
                        ps[:], xt_sb[:, d, 128 * tt:128 * (tt + 1)],
                        wv_sb[:, d, :],
                        start=(d == 0), stop=(d == ND - 1))
                nc.vector.tensor_tensor(
                    v_sb[:, tt, :, 0:HD],
                    ps.rearrange("p (h f) -> p h f", h=HPC),
                    bv_bc.rearrange("p (h f) -> p h f", h=HPC),
                    mybir.AluOpType.add)

            # per-phase A2A bounce buffers
            cc_in = [dram.tile([NCORES * FPC, PW[p]], BF16, name=f"cc_in{p}")
                     for p in range(len(PW))]
            cc_out = [dram.tile([NCORES * FPC, PW[p]], BF16, name=f"cc_out{p}")
                      for p in range(len(PW))]
            at_full = [acts.tile([128, NCORES, PW[p]], BF16, name=f"at_full{p}")
                       for p in range(len(PW))]

            def attention(p, b, qc, fillers=(), hp_div=False, o_delay=0):
                """One (batch, query-chunk) call for column phase p.

                fillers: list of (group_idx, emit_fn) -- extra PE work emitted
                right after that group's score matmuls so it executes in the
                exp-wait slack without delaying the ACT stream.
                """
                w = PW[p]
                kg = KG[p]
                ngrp = NKT // kg
                q0 = 2048 * b + 512 * qc + OFF[p]
                # h-stride = 512 f32 = one full PSUM bank per head: a
                # start=True matmul clears has_written for its whole bank.
                o_ps = o_ps_pool.tile([128, HPC, 512], F32, name="o_ps")
                # o_delay: hold back the first groups' o-chains (keeps early
                # scores from sitting behind V-DMA-gated o-matmuls in the PE
                # queue during the startup ramp); accumulation order preserved.
                held = []
                for g in range(ngrp):
                    s_ps = s_ps_pool.tile([128, HPC, kg, w], F32, name="s_ps")
                    for ki in range(kg):
                        k = g * kg + ki
                        k0 = 2048 * b + 128 * k
                        for h in range(HPC):
                            nc.tensor.matmul(
                                s_ps[:, h, ki, :],
                                kt_sb[64 * h:64 * (h + 1), k0:k0 + 128],
                                qt_sb[64 * h:64 * (h + 1), q0:q0 + w],
                                start=True, stop=True)
                    for (fg, fn) in fillers:
                        if fg == g:
                            fn()
                    p_t = p_pool.tile([128, HPC, kg, w], BF16, name="p_t")
                    nc.scalar.activation(
                        p_t[:], s_ps[:],
                        mybir.ActivationFunctionType.Exp, scale=SCALE)

                    def emit_o(g, p_t):
                        for ki in range(kg):
                            k = g * kg + ki
                            for h in range(HPC):
                                nc.tensor.matmul(
                                    o_ps[0:HD + 1, h, 0:w],
                                    v_sb[:, NKT * b + k, h, :],
                                    p_t[:, h, ki, :],
                                    start=(k == 0), stop=(k == NKT - 1))
                    if g < o_delay:
                        held.append((g, p_t))
                    else:
                        for (hg, hp) in held:
                            emit_o(hg, hp)
                        held = []
                        emit_o(g, p_t)
                j = NQC * b + qc
                from contextlib import nullcontext
                for h in range(HPC):
                    prio = tc.high_priority() if hp_div else nullcontext()
                    with prio:
                        # drain o out of PSUM first so the single o buffer
                        # frees before the next call's accumulation
                        ovs = div_pool.tile([HD + 1, w], F32, name="ovs")
                        nc.vector.tensor_copy(ovs[:], o_ps[0:HD + 1, h, 0:w])
                        den_sb = div_pool.tile([1, w], F32, name="den_sb")
                        nc.vector.tensor_copy(den_sb[:], ovs[HD:HD + 1, :])
                        recip = div_pool.tile([1, w], F32, name="recip")
                        nc.vector.reciprocal_approx_fast(recip[:], den_sb[:])
                        rb = div_pool.tile([HD, w], F32, name="rb")
                        nc.gpsimd.partition_broadcast(rb[:], recip[:])
                        avs = div_pool.tile([HD, w], BF16, name="avs")
                        nc.vector.tensor_tensor(
                            avs[:], ovs[0:HD, :], rb[:],
                            mybir.AluOpType.mult)
                        # gpsimd queue: the sync queue can head-of-line block
                        # for the full collective duration behind hoisted
                        # at_full loads -- the division path must not sit
                        # behind that, whatever the collective's speed
                        nc.gpsimd.dma_start(
                            out=cc_in[p][FPC * j + HD * h: FPC * j + HD * (h + 1), :],
                            in_=avs[:])

            def do_a2a(p):
                nc.gpsimd.collective_compute(
                    "AllToAll", mybir.AluOpType.bypass,
                    replica_groups=[list(range(NCORES))],
                    ins=[cc_in[p].opt()], outs=[cc_out[p].opt()])

            def load_at_full(p):
                for jj in range(NCORES):
                    nc.sync.dma_start(
                        out=at_full[p][:, jj, :],
                        in_=cc_out[p][FPC * jj:FPC * (jj + 1), :])

            def out_proj_m(p, m):
                w = PW[p]
                ps = mm_ps.tile([128, 512], F32, tag="mm_ps", name="f_ps")
                for d in range(ND):
                    nc.tensor.matmul(
                        ps[:, 0:w], wot_sb[:, d, 128 * m:128 * (m + 1)],
                        at_full[p][:, d, :],
                        start=(d == 0), stop=(d == ND - 1))
                fin = fin_pool.tile([128, w], F32, name="fin")
                nc.vector.tensor_scalar_add(fin[:], ps[:, 0:w], bo_sb[:, m:m + 1])
                q = nc.sync if m % 2 == 0 else nc.gpsimd
                q.dma_start(
                    out=out_t[128 * m:128 * (m + 1), OFF[p]:OFF[p] + w],
                    in_=fin[:])

            # ================= phase A (cols 0:256) =================
            # upfront: only tokens 0:256 of K and Q -- the minimum for the
            # first score group; everything else arrives just-in-time
            proj_qk_w(wk_sb, bk_sb, kt_sb, 0, 256)
            proj_qk_w(wq_sb, bq_sb, qt_sb, 0, 256)

            # call (0,0): K chunks / Q remainder / V tiles just-in-time
            attention(0, 0, 0, o_delay=4, fillers=[
                (0, lambda: (proj_qk_w(wk_sb, bk_sb, kt_sb, 256, 256),
                             proj_qk_w(wq_sb, bq_sb, qt_sb, 256, 256),
                             proj_v(0), proj_v(1))),
                (1, lambda: (proj_qk(wk_sb, bk_sb, kt_sb, 1),
                             proj_v(2), proj_v(3))),
                (2, lambda: (proj_qk(wk_sb, bk_sb, kt_sb, 2),
                             proj_v(4), proj_v(5))),
                (3, lambda: (proj_v(6), proj_v(7))),
                (4, lambda: (proj_qk(wk_sb, bk_sb, kt_sb, 3),
                             proj_v(8), proj_v(9))),
                (5, lambda: (proj_v(10), proj_v(11))),
                (6, lambda: (proj_v(12), proj_v(13), proj_v(14))),
                (7, lambda: (proj_v(15), proj_qk(wq_sb, bq_sb, qt_sb, 1))),
            ])
            # calls (0,1)-(0,3): spread batch-1 K/Q/V projections, with
            # Q/K chains split in half so no group slot exceeds its slack
            q2a, q2b = proj_qk_split(wq_sb, bq_sb, qt_sb, 2)
            k4a, k4b = proj_qk_split(wk_sb, bk_sb, kt_sb, 4)
            attention(0, 0, 1, fillers=[
                (0, q2a), (1, q2b),
                (2, lambda: (k4a(), proj_v(16))), (3, lambda: (k4b(), proj_v(17))),
                (4, lambda: proj_v(18)), (5, lambda: proj_v(19)),
                (6, lambda: proj_v(20)), (7, lambda: proj_v(21)),
            ])
            q3a, q3b = proj_qk_split(wq_sb, bq_sb, qt_sb, 3)
            k5a, k5b = proj_qk_split(wk_sb, bk_sb, kt_sb, 5)
            attention(0, 0, 2, fillers=[
                (0, q3a), (1, q3b),
                (2, lambda: (k5a(), proj_v(22))), (3, lambda: (k5b(), proj_v(23))),
                (4, lambda: proj_v(24)), (5, lambda: proj_v(25)),
                (6, lambda: proj_v(26)), (7, lambda: proj_v(27)),
            ])
            k6a, k6b = proj_qk_split(wk_sb, bk_sb, kt_sb, 6)
            k7a, k7b = proj_qk_split(wk_sb, bk_sb, kt_sb, 7)
            q4a, q4b = proj_qk_split(wq_sb, bq_sb, qt_sb, 4)
            attention(0, 0, 3, fillers=[
                (0, k6a), (1, k6b),
                (2, lambda: (k7a(), proj_v(28))), (3, lambda: (k7b(), proj_v(29))),
                (4, lambda: (q4a(), proj_v(30))), (5, lambda: (q4b(), proj_v(31))),
            ])
            q5a, q5b = proj_qk_split(wq_sb, bq_sb, qt_sb, 5)
            attention(0, 1, 0, fillers=[(0, q5a), (1, q5b)])
            q6a, q6b = proj_qk_split(wq_sb, bq_sb, qt_sb, 6)
            attention(0, 1, 1, fillers=[(0, q6a), (1, q6b)])
            q7a, q7b = proj_qk_split(wq_sb, bq_sb, qt_sb, 7)
            attention(0, 1, 2, fillers=[(0, q7a), (1, q7b)])
            attention(0, 1, 3)
            do_a2a(0)

            # ================= phase B (cols 256:384) =================
            # a2a(0) runs under the first ~half of this phase; out-projection
            # of phase A is emitted in the second half (after it is surely
            # complete) so the PE never head-of-line blocks on the collective.
            attention(1, 0, 0)
            attention(1, 0, 1)
            attention(1, 0, 2)
            attention(1, 0, 3)
            attention(1, 1, 0)
            attention(1, 1, 1)
            attention(1, 1, 2)
            attention(1, 1, 3)
            do_a2a(1)

            # ================= phase C (cols 384:512) =================
            load_at_full(0)
            attention(2, 0, 0, fillers=[
                (0, lambda: out_proj_m(0, 0)), (2, lambda: out_proj_m(0, 1)),
            ])
            attention(2, 0, 1, fillers=[
                (0, lambda: out_proj_m(0, 2)), (2, lambda: out_proj_m(0, 3)),
            ])
            attention(2, 0, 2, fillers=[
                (0, lambda: out_proj_m(0, 4)), (2, lambda: out_proj_m(0, 5)),
            ])
            attention(2, 0, 3, fillers=[
                (0, lambda: out_proj_m(0, 6)), (2, lambda: out_proj_m(0, 7)),
            ])
            load_at_full(1)
            attention(2, 1, 0, fillers=[
                (0, lambda: out_proj_m(1, 0)), (2, lambda: out_proj_m(1, 1)),
            ])
            attention(2, 1, 1, fillers=[
                (0, lambda: out_proj_m(1, 2)), (2, lambda: out_proj_m(1, 3)),
            ])
            attention(2, 1, 2, fillers=[
                (0, lambda: out_proj_m(1, 4)), (2, lambda: out_proj_m(1, 5)),
            ])
            attention(2, 1, 3, fillers=[
                (0, lambda: out_proj_m(1, 6)), (2, lambda: out_proj_m(1, 7)),
            ])
            do_a2a(2)

            # ================= exposed tail =================
            # keep the PE busy (HAM warm) across the final collective window
            tail_ps = mm_ps.tile([128, 512], F32, tag="mm_ps", name="tail_ps")
            for i in range(60):
                nc.tensor.matmul(tail_ps[:, 0:256], wmm_sb[:, 0:128],
                                 wmm_sb[:, 0:256],
                                 start=(i == 0), stop=(i == 59))
            for jj in range(NCORES):
                q = nc.sync if jj < 4 else nc.gpsimd
                q.dma_start(
                    out=at_full[2][:, jj, :],
                    in_=cc_out[2][FPC * jj:FPC * (jj + 1), :])
            for m in range(ND):
                out_proj_m(2, m)

    nc.compile()
    nc.m = get_hw_module(nc.m)
    return nc


_NC_CACHE = None


def _get_nc():
    global _NC_CACHE
    if _NC_CACHE is None:
        _NC_CACHE = build()
    return _NC_CACHE


def _make_in_maps(x, Wq, bq, Wk, bk, Wv, bv, Wo, bo):
    bf16 = ml_dtypes.bfloat16
    x = np.asarray(x, np.float32)
    xt = np.ascontiguousarray(x.reshape(T, D).T).astype(bf16)
    wot = np.ascontiguousarray(np.asarray(Wo, np.float32).T).astype(bf16)
    bo_col = np.asarray(bo, np.float32).reshape(D, 1)
    in_maps = []
    for c in range(NCORES):
        hs = slice(FPC * c, FPC * (c + 1))
        in_maps.append({
            "xt": xt,
            "wqt": np.ascontiguousarray(np.asarray(Wq, np.float32)[hs, :].T).astype(bf16),
            "wkt": np.ascontiguousarray(np.asarray(Wk, np.float32)[hs, :].T).astype(bf16),
            "wvt": np.ascontiguousarray(np.asarray(Wv, np.float32)[hs, :].T).astype(bf16),
            "wot": wot,
            "bq": np.asarray(bq, np.float32)[hs].reshape(FPC, 1),
            "bk": np.asarray(bk, np.float32)[hs].reshape(FPC, 1),
            "bv_row": np.asarray(bv, np.float32)[hs].reshape(1, FPC).astype(bf16),
            "bo": bo_col,
        })
    return in_maps


def run_on_hw(in_maps, trace=False):
    nc = _get_nc()
    return run_bass_kernel_spmd(nc, in_maps, list(range(NCORES)), trace=trace)


def _assemble(results):
    out = np.empty((T, D), np.float32)
    for c in range(NCORES):
        out[TPC * c:TPC * (c + 1), :] = results[c]["out_t"].T
    return out.reshape(B, S, D)


def kernel(x, Wq, bq, Wk, bk, Wv, bv, Wo, bo):
    in_maps = _make_in_maps(x, Wq, bq, Wk, bk, Wv, bv, Wo, bo)
    res = run_on_hw(in_maps, trace=False)
    return _assemble(res.results)


# revision 37
# speedup vs baseline: 1.3205x; 1.2548x over previous
"""Multi-head attention (B=2, S=2048, D=1024, H=16) on 8 TRN2 NeuronCores.

Sharding: 8-way tensor-parallel over heads (2 heads/core), Megatron-style.
Each core:
  - holds X^T (feature-major) for all 4096 flat tokens (bf16),
  - computes Q^T/K^T (feature-major) and V (token-major) for its 2 heads,
  - computes scores S^T = K_h Q_h^T per k-tile with K=64 matmuls row-packed
    into the 128x128 PE array (one per head),
  - softmax: exp on ScalarE (scale=1/8 folded in, no max-subtraction --
    scores are ~N(0, 1/3) so exp is safe), denominator via a ones-column
    appended to V (M=65 matmul), division via fast reciprocal +
    partition_broadcast,
  - AllToAll over all 8 cores redistributes attention outputs from
    head-sharded to token-sharded (each core ends with all 1024 attention
    features for its 512-token chunk),
  - out-projection (full Wo^T) + bias for its token chunk.
Host only pre-transposes/slices inputs and re-assembles output slices.

v2 schedule: the query axis is processed in THREE column phases of widths
[256, 128, 128].  Phase p's AllToAll and out-projection are hidden under
phase p+1's attention compute; only the final 256KB AllToAll plus a
128-column out-projection are exposed at the tail.  The ScalarE exp stream
(~171us total) is the pacing engine, so:
  - the scalar queue carries only the K-weight DMAs (which gate the first
    exp anyway) and then exp instructions exclusively,
  - QKV projection matmuls for batch 1 are emitted interleaved between
    batch-0 attention groups so the PE fills exp-wait slack instead of
    stalling the ACT stream at the batch seam,
  - exp granularity is 1024 free elements per ACTIVATE in every phase
    (phases B/C group 4 k-tiles per exp to amortize the ~480ns
    per-instruction ACT overhead).

PSUM note: a start=True matmul clears has_written bits for its WHOLE 2KB
bank, so the two heads' o-accumulation chains live in separate banks
(h-stride = 512 f32) and the o tile is drained to SBUF immediately after
the accumulation completes (high priority) so a single o buffer suffices.
"""
import numpy as np
import ml_dtypes

import concourse.bass as bass
import concourse.bacc as bacc
import concourse.tile as tile
import concourse.mybir as mybir
from concourse.bass_utils import run_bass_kernel_spmd
from concourse.bass_interp import get_hw_module

NCORES = 8
B, S, D = 2, 2048, 1024
H, HD = 16, 64
T = B * S                 # 4096 flat tokens
HPC = H // NCORES         # 2 heads per core
FPC = HPC * HD            # 128 features per core
TPC = T // NCORES         # 512-token output chunk per core
ND = D // 128             # 8 d-tiles
NTT = T // 128            # 32 token-tiles
NKT = S // 128            # 16 key tiles per batch
NQC = 4                   # 512-token query chunks per batch
SCALE = 1.0 / float(np.sqrt(HD))

PW = [256, 128, 128]      # query-column width per phase
OFF = [0, 256, 384]       # column offset per phase
KG = [2, 4, 4]            # k-tiles grouped per exp (free = 2*KG*PW = 1024)

BF16 = mybir.dt.bfloat16
F32 = mybir.dt.float32


def build():
    nc = bacc.Bacc("TRN2", target_bir_lowering=False, debug=False,
                   num_devices=NCORES)
    xt = nc.dram_tensor("xt", [D, T], BF16, kind="ExternalInput").ap()
    wqt = nc.dram_tensor("wqt", [D, FPC], BF16, kind="ExternalInput").ap()
    wkt = nc.dram_tensor("wkt", [D, FPC], BF16, kind="ExternalInput").ap()
    wvt = nc.dram_tensor("wvt", [D, FPC], BF16, kind="ExternalInput").ap()
    wot = nc.dram_tensor("wot", [D, D], BF16, kind="ExternalInput").ap()
    bq = nc.dram_tensor("bq", [FPC, 1], F32, kind="ExternalInput").ap()
    bk = nc.dram_tensor("bk", [FPC, 1], F32, kind="ExternalInput").ap()
    bv_row = nc.dram_tensor("bv_row", [1, FPC], BF16, kind="ExternalInput").ap()
    bo = nc.dram_tensor("bo", [D, 1], F32, kind="ExternalInput").ap()
    out_t = nc.dram_tensor("out_t", [D, TPC], F32, kind="ExternalOutput").ap()

    with tile.TileContext(nc) as tc:
        with (
            tc.tile_pool(name="wts", bufs=1) as wts,
            tc.tile_pool(name="acts", bufs=1) as acts,
            tc.tile_pool(name="p_pool", bufs=6) as p_pool,
            tc.tile_pool(name="div_pool", bufs=10) as div_pool,
            tc.tile_pool(name="fin_pool", bufs=2) as fin_pool,
            tc.tile_pool(name="s_ps_pool", bufs=2, space="PSUM") as s_ps_pool,
            tc.tile_pool(name="o_ps_pool", bufs=1, space="PSUM") as o_ps_pool,
            tc.tile_pool(name="mm_ps", bufs=2, space="PSUM") as mm_ps,
            tc.tile_pool(name="dram", bufs=1, space="DRAM") as dram,
        ):
            # ---- ACT warm-up: table load + one tiny exp before anything else
            # ---- on the scalar queue.
            warm_sb = wts.tile([1, 8], F32, name="warm_sb")
            nc.vector.memset(warm_sb[:], 0.0)
            nc.scalar.activation(warm_sb[:], warm_sb[:],
                                 mybir.ActivationFunctionType.Exp, scale=1.0)

            # ---- DMAs, priority order: K-path data first (wk, bk, xt cols
            # ---- 0:1024), then Q weights, V weights, rest of batch 0,
            # ---- batch 1, Wo last.
            xt_sb = acts.tile([128, ND, T], BF16, name="xt_sb")
            wq_sb = wts.tile([128, ND, FPC], BF16, name="wq_sb")
            wk_sb = wts.tile([128, ND, FPC], BF16, name="wk_sb")
            wv_sb = wts.tile([128, ND, FPC], BF16, name="wv_sb")
            bq_sb = wts.tile([FPC, 1], F32, name="bq_sb")
            bk_sb = wts.tile([FPC, 1], F32, name="bk_sb")
            bv_row_sb = wts.tile([1, FPC], BF16, name="bv_row_sb")

            # first xt wave (tokens 0:256 gate the first K/Q projection)
            # split over sync + gpsimd so both DMA paths pull in parallel;
            # K/Q weights on the scalar queue (they gate the first exp anyway)
            for d in range(ND):
                q = nc.sync if d < 4 else nc.gpsimd
                q.dma_start(out=xt_sb[:, d, 0:256], in_=xt[128 * d:128 * (d + 1), 0:256])
            for d in range(ND):
                q = nc.sync if d < 4 else nc.gpsimd
                q.dma_start(out=xt_sb[:, d, 256:512], in_=xt[128 * d:128 * (d + 1), 256:512])
            for d in range(ND):
                nc.scalar.dma_start(out=wk_sb[:, d, :], in_=wkt[128 * d:128 * (d + 1), :])
            nc.scalar.dma_start(out=bk_sb[:], in_=bk[:])
            # pre-warm the collective path: a tiny dummy AllToAll absorbs the
            # one-time ~50us first-collective setup cost
            cc_wu_in = dram.tile([NCORES, 64], BF16, name="cc_wu_in")
            cc_wu_out = dram.tile([NCORES, 64], BF16, name="cc_wu_out")
            nc.gpsimd.collective_compute(
                "AllToAll", mybir.AluOpType.bypass,
                replica_groups=[list(range(NCORES))],
                ins=[cc_wu_in.opt()], outs=[cc_wu_out.opt()])
            for d in range(ND):
                nc.gpsimd.dma_start(out=wq_sb[:, d, :], in_=wqt[128 * d:128 * (d + 1), :])
            nc.gpsimd.dma_start(out=bq_sb[:], in_=bq[:])
            for d in range(ND):
                nc.sync.dma_start(out=wv_sb[:, d, :], in_=wvt[128 * d:128 * (d + 1), :])
            nc.sync.dma_start(out=bv_row_sb[:], in_=bv_row[:])
            for d in range(ND):
                q = nc.sync if d < 4 else nc.gpsimd
                q.dma_start(out=xt_sb[:, d, 512:1024], in_=xt[128 * d:128 * (d + 1), 512:1024])
            bv_bc = wts.tile([128, FPC], BF16, name="bv_bc")
            nc.gpsimd.partition_broadcast(bv_bc[:], bv_row_sb[:])
            for d in range(ND):
                q = nc.sync if d < 4 else nc.gpsimd
                q.dma_start(out=xt_sb[:, d, 1024:2048], in_=xt[128 * d:128 * (d + 1), 1024:2048])
            # batch 1
            for d in range(ND):
                q = nc.sync if d < 4 else nc.gpsimd
                q.dma_start(out=xt_sb[:, d, S:T], in_=xt[128 * d:128 * (d + 1), S:T])
            # out-projection weights (needed from ~mid-run)
            wot_sb = wts.tile([128, ND, D], BF16, name="wot_sb")
            bo_sb = wts.tile([128, ND], F32, name="bo_sb")
            for d in range(ND):
                nc.gpsimd.dma_start(out=wot_sb[:, d, :], in_=wot[128 * d:128 * (d + 1), :])
            for m in range(ND):
                nc.gpsimd.dma_start(out=bo_sb[:, m:m + 1], in_=bo[128 * m:128 * (m + 1), :])

            # ---- warm up the PE HAM clock gate (needs ~3.4us of sustained
            # ---- matmul activity to unthrottle 1.2 -> 2.4 GHz)
            wmm_sb = wts.tile([128, 512], BF16, name="wmm_sb")
            nc.vector.memset(wmm_sb[:], 0.0)
            warm_ps = mm_ps.tile([128, 512], F32, tag="mm_ps", name="warm_ps")
            for i in range(24):
                nc.tensor.matmul(warm_ps[:], wmm_sb[:, 0:128], wmm_sb[:, 0:512],
                                 start=(i == 0), stop=(i == 23))
            nc.vector.tensor_copy(wmm_sb[0:1, 0:1], warm_ps[0:1, 0:1])

            qt_sb = acts.tile([FPC, T], BF16, name="qt_sb")
            kt_sb = acts.tile([FPC, T], BF16, name="kt_sb")
            v_sb = acts.tile([128, NTT, HPC, HD + 1], BF16, name="v_sb")
            nc.vector.memset(v_sb[:, :, :, HD:HD + 1], 1.0)

            def proj_qk(w_sb, b_sb, dst, tch):
                ps = mm_ps.tile([128, 512], F32, tag="mm_ps", name="proj_ps")
                for d in range(ND):
                    nc.tensor.matmul(
                        ps[:], w_sb[:, d, :],
                        xt_sb[:, d, 512 * tch:512 * (tch + 1)],
                        start=(d == 0), stop=(d == ND - 1))
                nc.vector.tensor_scalar_add(
                    dst[:, 512 * tch:512 * (tch + 1)], ps[:], b_sb[:])

            def proj_qk_w(w_sb, b_sb, dst, t0, tw):
                # narrow-width projection for startup ramp (tokens t0:t0+tw)
                ps = mm_ps.tile([128, 512], F32, tag="mm_ps", name="proj_ps")
                for d in range(ND):
                    nc.tensor.matmul(
                        ps[:, 0:tw], w_sb[:, d, :],
                        xt_sb[:, d, t0:t0 + tw],
                        start=(d == 0), stop=(d == ND - 1))
                nc.vector.tensor_scalar_add(
                    dst[:, t0:t0 + tw], ps[:, 0:tw], b_sb[:])

            def proj_qk_split(w_sb, b_sb, dst, tch):
                # returns two half-chain emitters so a projection can spread
                # over two attention groups' exp-wait slack
                ps = mm_ps.tile([128, 512], F32, tag="mm_ps", name="proj_ps")

                def part1():
                    for d in range(4):
                        nc.tensor.matmul(
                            ps[:], w_sb[:, d, :],
                            xt_sb[:, d, 512 * tch:512 * (tch + 1)],
                            start=(d == 0), stop=False)

                def part2():
                    for d in range(4, ND):
                        nc.tensor.matmul(
                            ps[:], w_sb[:, d, :],
                            xt_sb[:, d, 512 * tch:512 * (tch + 1)],
                            start=False, stop=(d == ND - 1))
                    nc.vector.tensor_scalar_add(
                        dst[:, 512 * tch:512 * (tch + 1)], ps[:], b_sb[:])
                return part1, part2

            def proj_v(tt):
                ps = mm_ps.tile([128, FPC], F32, tag="mm_ps", name="v_ps")
                for d in range(ND):
                    nc.tensor.matmul(
                        ps[:], xt_sb[:, d, 128 * tt:128 * (tt + 1)],
                        wv_sb[:, d, :],
                        start=(d == 0), stop=(d == ND - 1))
                nc.vector.tensor_tensor(
                    v_sb[:, tt, :, 0:HD],
                    ps.rearrange("p (h f) -> p h f", h=HPC),
                    bv_bc.rearrange("p (h f) -> p h f", h=HPC),
                    mybir.AluOpType.add)

            # per-phase A2A bounce buffers
            cc_in = [dram.tile([NCORES * FPC, PW[p]], BF16, name=f"cc_in{p}")
                     for p in range(len(PW))]
            cc_out = [dram.tile([NCORES * FPC, PW[p]], BF16, name=f"cc_out{p}")
                      for p in range(len(PW))]
            at_full = [acts.tile([128, NCORES, PW[p]], BF16, name=f"at_full{p}")
                       for p in range(len(PW))]

            def attention(p, b, qc, fillers=(), hp_div=False, o_delay=0):
                """One (batch, query-chunk) call for column phase p.

                fillers: list of (group_idx, emit_fn) -- extra PE work emitted
                right after that group's score matmuls so it executes in the
                exp-wait slack without delaying the ACT stream.
                """
                w = PW[p]
                kg = KG[p]
                ngrp = NKT // kg
                q0 = 2048 * b + 512 * qc + OFF[p]
                # h-stride = 512 f32 = one full PSUM bank per head: a
                # start=True matmul clears has_written for its whole bank.
                o_ps = o_ps_pool.tile([128, HPC, 512], F32, name="o_ps")
                # o_delay: hold back the first groups' o-chains (keeps early
                # scores from sitting behind V-DMA-gated o-matmuls in the PE
                # queue during the startup ramp); accumulation order preserved.
                held = []
                for g in range(ngrp):
                    s_ps = s_ps_pool.tile([128, HPC, kg, w], F32, name="s_ps")
                    for ki in range(kg):
                        k = g * kg + ki
                        k0 = 2048 * b + 128 * k
                        for h in range(HPC):
                            nc.tensor.matmul(
                                s_ps[:, h, ki, :],
                                kt_sb[64 * h:64 * (h + 1), k0:k0 + 128],
                                qt_sb[64 * h:64 * (h + 1), q0:q0 + w],
                                start=True, stop=True)
                    for (fg, fn) in fillers:
                        if fg == g:
                            fn()
                    p_t = p_pool.tile([128, HPC, kg, w], BF16, name="p_t")
                    nc.scalar.activation(
                        p_t[:], s_ps[:],
                        mybir.ActivationFunctionType.Exp, scale=SCALE)

                    def emit_o(g, p_t):
                        for ki in range(kg):
                            k = g * kg + ki
                            for h in range(HPC):
                                nc.tensor.matmul(
                                    o_ps[0:HD + 1, h, 0:w],
                                    v_sb[:, NKT * b + k, h, :],
                                    p_t[:, h, ki, :],
                                    start=(k == 0), stop=(k == NKT - 1))
                    if g < o_delay:
                        held.append((g, p_t))
                    else:
                        for (hg, hp) in held:
                            emit_o(hg, hp)
                        held = []
                        emit_o(g, p_t)
                j = NQC * b + qc
                from contextlib import nullcontext
                for h in range(HPC):
                    prio = tc.high_priority() if hp_div else nullcontext()
                    with prio:
                        # drain o out of PSUM first so the single o buffer
                        # frees before the next call's accumulation
                        ovs = div_pool.tile([HD + 1, w], F32, name="ovs")
                        nc.vector.tensor_copy(ovs[:], o_ps[0:HD + 1, h, 0:w])
                        den_sb = div_pool.tile([1, w], F32, name="den_sb")
                        nc.vector.tensor_copy(den_sb[:], ovs[HD:HD + 1, :])
                        recip = div_pool.tile([1, w], F32, name="recip")
                        nc.vector.reciprocal_approx_fast(recip[:], den_sb[:])
                        rb = div_pool.tile([HD, w], F32, name="rb")
                        nc.gpsimd.partition_broadcast(rb[:], recip[:])
                        avs = div_pool.tile([HD, w], BF16, name="avs")
                        nc.vector.tensor_tensor(
                            avs[:], ovs[0:HD, :], rb[:],
                            mybir.AluOpType.mult)
                        # gpsimd queue: the sync queue can head-of-line block
                        # for the full collective duration behind hoisted
                        # at_full loads -- the division path must not sit
                        # behind that, whatever the collective's speed
                        nc.gpsimd.dma_start(
                            out=cc_in[p][FPC * j + HD * h: FPC * j + HD * (h + 1), :],
                            in_=avs[:])

            def do_a2a(p):
                nc.gpsimd.collective_compute(
                    "AllToAll", mybir.AluOpType.bypass,
                    replica_groups=[list(range(NCORES))],
                    ins=[cc_in[p].opt()], outs=[cc_out[p].opt()])

            def load_at_full(p):
                for jj in range(NCORES):
                    nc.sync.dma_start(
                        out=at_full[p][:, jj, :],
                        in_=cc_out[p][FPC * jj:FPC * (jj + 1), :])

            def out_proj_m(p, m):
                w = PW[p]
                ps = mm_ps.tile([128, 512], F32, tag="mm_ps", name="f_ps")
                for d in range(ND):
                    nc.tensor.matmul(
                        ps[:, 0:w], wot_sb[:, d, 128 * m:128 * (m + 1)],
                        at_full[p][:, d, :],
                        start=(d == 0), stop=(d == ND - 1))
                fin = fin_pool.tile([128, w], F32, name="fin")
                nc.vector.tensor_scalar_add(fin[:], ps[:, 0:w], bo_sb[:, m:m + 1])
                q = nc.sync if m % 2 == 0 else nc.gpsimd
                q.dma_start(
                    out=out_t[128 * m:128 * (m + 1), OFF[p]:OFF[p] + w],
                    in_=fin[:])

            # ================= phase A (cols 0:256) =================
            # upfront: only tokens 0:256 of K and Q -- the minimum for the
            # first score group; everything else arrives just-in-time
            proj_qk_w(wk_sb, bk_sb, kt_sb, 0, 256)
            proj_qk_w(wq_sb, bq_sb, qt_sb, 0, 256)

            # call (0,0): K chunks / Q remainder / V tiles just-in-time
            attention(0, 0, 0, o_delay=4, fillers=[
                (0, lambda: (proj_qk_w(wk_sb, bk_sb, kt_sb, 256, 256),
                             proj_v(0), proj_v(1))),
                (1, lambda: (proj_qk(wk_sb, bk_sb, kt_sb, 1),
                             proj_v(2), proj_v(3))),
                (2, lambda: (proj_qk(wk_sb, bk_sb, kt_sb, 2),
                             proj_v(4), proj_v(5))),
                (3, lambda: (proj_v(6), proj_v(7))),
                (4, lambda: (proj_qk(wk_sb, bk_sb, kt_sb, 3),
                             proj_v(8), proj_v(9))),
                (5, lambda: (proj_v(10), proj_v(11))),
                (6, lambda: (proj_v(12), proj_v(13), proj_v(14))),
                (7, lambda: (proj_v(15),
                             proj_qk_w(wq_sb, bq_sb, qt_sb, 512, 256))),
            ])
            # calls (0,1)-(0,3): spread batch-1 K/Q/V projections, with
            # Q/K chains split in half so no group slot exceeds its slack
            q2a = lambda: proj_qk_w(wq_sb, bq_sb, qt_sb, 1024, 256)
            k4a, k4b = proj_qk_split(wk_sb, bk_sb, kt_sb, 4)
            attention(0, 0, 1, fillers=[
                (0, q2a),
                (2, lambda: (k4a(), proj_v(16))), (3, lambda: (k4b(), proj_v(17))),
                (4, lambda: proj_v(18)), (5, lambda: proj_v(19)),
                (6, lambda: proj_v(20)), (7, lambda: proj_v(21)),
            ])
            q3a = lambda: proj_qk_w(wq_sb, bq_sb, qt_sb, 1536, 256)
            k5a, k5b = proj_qk_split(wk_sb, bk_sb, kt_sb, 5)
            attention(0, 0, 2, fillers=[
                (0, q3a),
                (2, lambda: (k5a(), proj_v(22))), (3, lambda: (k5b(), proj_v(23))),
                (4, lambda: proj_v(24)), (5, lambda: proj_v(25)),
                (6, lambda: proj_v(26)), (7, lambda: proj_v(27)),
            ])
            k6a, k6b = proj_qk_split(wk_sb, bk_sb, kt_sb, 6)
            k7a, k7b = proj_qk_split(wk_sb, bk_sb, kt_sb, 7)
            q4a = lambda: proj_qk_w(wq_sb, bq_sb, qt_sb, 2048, 256)
            attention(0, 0, 3, fillers=[
                (0, k6a), (1, k6b),
                (2, lambda: (k7a(), proj_v(28))), (3, lambda: (k7b(), proj_v(29))),
                (4, lambda: (q4a(), proj_v(30))), (5, lambda: proj_v(31)),
            ])
            attention(0, 1, 0, fillers=[
                (0, lambda: proj_qk_w(wq_sb, bq_sb, qt_sb, 2560, 256))])
            attention(0, 1, 1, fillers=[
                (0, lambda: proj_qk_w(wq_sb, bq_sb, qt_sb, 3072, 256))])
            attention(0, 1, 2, fillers=[
                (0, lambda: proj_qk_w(wq_sb, bq_sb, qt_sb, 3584, 256))])
            attention(0, 1, 3, fillers=[
                (0, lambda: proj_qk_w(wq_sb, bq_sb, qt_sb, 256, 256))])
            do_a2a(0)

            # ================= phase B (cols 256:384) =================
            # a2a(0) runs under the first ~half of this phase; out-projection
            # of phase A is emitted in the second half (after it is surely
            # complete) so the PE never head-of-line blocks on the collective.
            attention(1, 0, 0, fillers=[
                (0, lambda: proj_qk_w(wq_sb, bq_sb, qt_sb, 768, 256))])
            attention(1, 0, 1, fillers=[
                (0, lambda: proj_qk_w(wq_sb, bq_sb, qt_sb, 1280, 256))])
            attention(1, 0, 2, fillers=[
                (0, lambda: proj_qk_w(wq_sb, bq_sb, qt_sb, 1792, 256))])
            attention(1, 0, 3, fillers=[
                (0, lambda: proj_qk_w(wq_sb, bq_sb, qt_sb, 2304, 256))])
            attention(1, 1, 0, fillers=[
                (0, lambda: proj_qk_w(wq_sb, bq_sb, qt_sb, 2816, 256))])
            attention(1, 1, 1, fillers=[
                (0, lambda: proj_qk_w(wq_sb, bq_sb, qt_sb, 3328, 256))])
            attention(1, 1, 2, fillers=[
                (0, lambda: proj_qk_w(wq_sb, bq_sb, qt_sb, 3840, 256))])
            attention(1, 1, 3)
            do_a2a(1)

            # ================= phase C (cols 384:512) =================
            load_at_full(0)
            attention(2, 0, 0, fillers=[
                (0, lambda: out_proj_m(0, 0)), (2, lambda: out_proj_m(0, 1)),
            ])
            attention(2, 0, 1, fillers=[
                (0, lambda: out_proj_m(0, 2)), (2, lambda: out_proj_m(0, 3)),
            ])
            attention(2, 0, 2, fillers=[
                (0, lambda: out_proj_m(0, 4)), (2, lambda: out_proj_m(0, 5)),
            ])
            attention(2, 0, 3, fillers=[
                (0, lambda: out_proj_m(0, 6)), (2, lambda: out_proj_m(0, 7)),
            ])
            load_at_full(1)
            attention(2, 1, 0, fillers=[
                (0, lambda: out_proj_m(1, 0)), (2, lambda: out_proj_m(1, 1)),
            ])
            attention(2, 1, 1, fillers=[
                (0, lambda: out_proj_m(1, 2)), (2, lambda: out_proj_m(1, 3)),
            ])
            attention(2, 1, 2, fillers=[
                (0, lambda: out_proj_m(1, 4)), (2, lambda: out_proj_m(1, 5)),
            ])
            attention(2, 1, 3, fillers=[
                (0, lambda: out_proj_m(1, 6)), (2, lambda: out_proj_m(1, 7)),
            ])
            do_a2a(2)

            # ================= exposed tail =================
            # keep the PE busy (HAM warm) across the final collective window
            tail_ps = mm_ps.tile([128, 512], F32, tag="mm_ps", name="tail_ps")
            for i in range(60):
                nc.tensor.matmul(tail_ps[:, 0:256], wmm_sb[:, 0:128],
                                 wmm_sb[:, 0:256],
                                 start=(i == 0), stop=(i == 59))
            for jj in range(NCORES):
                q = nc.sync if jj < 4 else nc.gpsimd
                q.dma_start(
                    out=at_full[2][:, jj, :],
                    in_=cc_out[2][FPC * jj:FPC * (jj + 1), :])
            for m in range(ND):
                out_proj_m(2, m)

    nc.compile()
    nc.m = get_hw_module(nc.m)
    return nc


_NC_CACHE = None


def _get_nc():
    global _NC_CACHE
    if _NC_CACHE is None:
        _NC_CACHE = build()
    return _NC_CACHE


def _make_in_maps(x, Wq, bq, Wk, bk, Wv, bv, Wo, bo):
    bf16 = ml_dtypes.bfloat16
    x = np.asarray(x, np.float32)
    xt = np.ascontiguousarray(x.reshape(T, D).T).astype(bf16)
    wot = np.ascontiguousarray(np.asarray(Wo, np.float32).T).astype(bf16)
    bo_col = np.asarray(bo, np.float32).reshape(D, 1)
    in_maps = []
    for c in range(NCORES):
        hs = slice(FPC * c, FPC * (c + 1))
        in_maps.append({
            "xt": xt,
            "wqt": np.ascontiguousarray(np.asarray(Wq, np.float32)[hs, :].T).astype(bf16),
            "wkt": np.ascontiguousarray(np.asarray(Wk, np.float32)[hs, :].T).astype(bf16),
            "wvt": np.ascontiguousarray(np.asarray(Wv, np.float32)[hs, :].T).astype(bf16),
            "wot": wot,
            "bq": np.asarray(bq, np.float32)[hs].reshape(FPC, 1),
            "bk": np.asarray(bk, np.float32)[hs].reshape(FPC, 1),
            "bv_row": np.asarray(bv, np.float32)[hs].reshape(1, FPC).astype(bf16),
            "bo": bo_col,
        })
    return in_maps


def run_on_hw(in_maps, trace=False):
    nc = _get_nc()
    return run_bass_kernel_spmd(nc, in_maps, list(range(NCORES)), trace=trace)


def _assemble(results):
    out = np.empty((T, D), np.float32)
    for c in range(NCORES):
        out[TPC * c:TPC * (c + 1), :] = results[c]["out_t"].T
    return out.reshape(B, S, D)


def kernel(x, Wq, bq, Wk, bk, Wv, bv, Wo, bo):
    in_maps = _make_in_maps(x, Wq, bq, Wk, bk, Wv, bv, Wo, bo)
    res = run_on_hw(in_maps, trace=False)
    return _assemble(res.results)
